# revision 38
# baseline (speedup 1.0000x reference)
"""Trainium2 Bass kernel for nn_LocalEnhancementModule (8-core SPMD, data-parallel over batch).

Per-sample computation (B=16, P=256 patches, D=4096, E=512):
    p      = patchify(x)                       [P, D]
    theta  = p @ theta_w + theta_b             [P, E]
    f      = p @ f_w + f_b                     [P, E]
    wgt    = softmax(theta @ f.T, axis=-1)     [P, P]
    g      = p @ g_w + g_b                     [P, D]
    out    = unpatchify(wgt[:,None,:] * g.reshape(P,C,P)) * scale + x

Sharding: 2 samples per core (PP=512 patch rows).

Precision/schedule: theta/f run in fp16 (the softmax scores need ~fp16
accuracy; measured fp8 scores push rel-err past tolerance). The dominant
g projection runs in fp8-e4m3 with MatmulPerfMode.DoubleRow (2 fp8
weights per PE cell, 256-deep contraction per matmul, ~2x throughput).
p is cast fp16->fp8 on-device (x16 scaling), g_w is quantized host-side
(x512 scaling, clipped to +-240); the 1/8192 unfold is folded into the
softmax reciprocal. Residual and output ride fp16 DMA (x dominates the
output norm; fp16 rounding is ~2e-4 rel). PSUM accumulates fp32
throughout.

DMA: k-quad interleaved layouts give 2-4KB contiguous rows per transfer.
Traffic is split over three rings: sync (tf even-k, gw8 even rounds),
gpsimd (tf odd-k, gw8 odd rounds), scalar (p tiles, residual, output).
"""

import sys
import numpy as np

try:
    import concourse.bacc as bacc
except ImportError:  # pragma: no cover
    for _p in ("/opt/trn_rl_repo", "/root/.axon_site/_ro/trn_rl_repo"):
        if _p not in sys.path:
            sys.path.append(_p)
    import concourse.bacc as bacc
import concourse.mybir as mybir
import concourse.tile as tile
from concourse.bass_utils import run_bass_kernel_spmd

NCORES = 8
B, C, H, W = 16, 16, 256, 256
NPS, PH, PW = 16, 16, 16
P = NPS * NPS            # 256 patches
D = C * PH * PW          # 4096
E = 512
SPC = B // NCORES        # 2 samples per core
PP = SPC * P             # 512 patch rows per core
KT = D // 128            # 32 contraction tiles of 128
KT4 = D // 512           # 8 k-quad tiles (4 x 128)
ET = E // 128            # 4 embedding chunks
DCH = D // 512           # 8 column rounds for g
DP = DCH // 2            # 4 column-pair rounds (1024-wide output writes)
GRP = [(s, pc) for s in range(SPC) for pc in range(2)]

SF_P = 16.0              # fp8 scale for p
SF_G = 512.0             # fp8 scale for g_w
UNSCALE = 1.0 / (SF_P * SF_G)

F32 = mybir.dt.float32
F16 = mybir.dt.float16
F8 = mybir.dt.float8e4
DR = mybir.MatmulPerfMode.DoubleRow

_built = {}
LAST_RESULTS = None  # stashed BassKernelResults for test harness introspection


def _build(with_tb, with_fb, with_gb):
    key = (with_tb, with_fb, with_gb)
    if key in _built:
        return _built[key]

    nc = bacc.Bacc("TRN2", num_devices=NCORES, debug=False)
    # ptq: pT fp16, k-quad interleaved: row kq*128+part, col ks*PP+pp,
    #      element = pT[k=kq*512+ks*128+part, pp]
    ptq_d = nc.dram_tensor("ptq", [KT4 * 128, 4 * PP], F16, kind="ExternalInput").ap()
    pnat_d = nc.dram_tensor("pnat", [PP, D], F16, kind="ExternalInput").ap()
    # tf: concat(theta_w | f_w) columns, k-quad interleaved like ptq:
    # row kq*128+part, col ks*1024 + wcol; element = tf[k=kq*512+ks*128+part, wcol]
    tf_d = nc.dram_tensor("tf", [KT4 * 128, 4 * 2 * E], F16, kind="ExternalInput").ap()
    # gw8: fp8 g_w * scale * SF_G; row kq*128+part, col dch*2048+ks*512+n,
    #      element = gw[k=kq*512+ks*128+part, d=dch*512+n]
    gw8_d = nc.dram_tensor("gw8", [KT4 * 128, DCH * 2048], F8, kind="ExternalInput").ap()
    tb_d = nc.dram_tensor("tb", [E, 1], F32, kind="ExternalInput").ap() if with_tb else None
    fb_d = nc.dram_tensor("fb", [E, 1], F32, kind="ExternalInput").ap() if with_fb else None
    gb_d = nc.dram_tensor("gb", [1, D], F32, kind="ExternalInput").ap() if with_gb else None
    out_d = nc.dram_tensor("out", [PP, D], F16, kind="ExternalOutput").ap()

    with tile.TileContext(nc) as tc:
        with tc.tile_pool(name="persist", bufs=1) as pp_, \
             tc.tile_pool(name="ptstream", bufs=8) as pqp, \
             tc.tile_pool(name="wstream", bufs=1) as wp, \
             tc.tile_pool(name="gstream", bufs=10) as gp, \
             tc.tile_pool(name="pnstream", bufs=6) as pnp, \
             tc.tile_pool(name="enh", bufs=6) as ep, \
             tc.tile_pool(name="sm", bufs=4) as smp:

            # PE warmup: ~48 trivial matmuls during the initial DMA wait so
            # the HAM clock-gate reaches K=8/8 before the first real matmul
            with tc.tile_pool(name="warm", bufs=1) as wmp, \
                 tc.tile_pool(name="psW", bufs=1, space="PSUM") as psW:
                wt_warm = wmp.tile([128, 128], F16, name="wt_warm", tag="warm")
                nc.vector.memset(wt_warm[:, :], 0.0)
                ps_warm = psW.tile([128, 128], F32, name="ps_warm", tag="psw")
                for _ in range(36):
                    nc.tensor.matmul(ps_warm[:, :], wt_warm[:, :], wt_warm[:, :],
                                     start=True, stop=True)

            bias_sb = {0: [], 1: []}
            for wi, bd in ((0, tb_d), (1, fb_d)):
                if bd is None:
                    continue
                for e in range(ET):
                    bt = pp_.tile([128, 1], F32, name=f"bias_{wi}_{e}", tag=f"bias_{wi}_{e}")
                    nc.scalar.dma_start(out=bt[:, :], in_=bd[e * 128:(e + 1) * 128, :])
                    bias_sb[wi].append(bt)
            gb_sb = None
            if gb_d is not None:
                gb_sb = pp_.tile([1, D], F32, name="gb_sb", tag="gb_sb")
                nc.scalar.dma_start(out=gb_sb[:, :], in_=gb_d[:, :])

            # ---- theta / f projections (fp16), k-outer into 8 PSUM banks ----
            pt8_sb = []
            with tc.tile_pool(name="psA", bufs=1, space="PSUM") as psA:
                ps_attn = {}
                for wi in (0, 1):
                    for e in range(ET):
                        ps_attn[(wi, e)] = psA.tile([128, PP], F32,
                                                    name=f"ps_attn_{wi}_{e}",
                                                    tag=f"attn_{wi}_{e}")
                for kq in range(KT4):
                    # per-ks descriptors + persistent tiles: fine-grained
                    # arrival keeps PE catch-up stalls under the 3.4us HAM
                    # re-throttle window, and no buffer-reuse WAR waits (a
                    # waiting DMA trigger head-of-line blocks its engine)
                    ptq_t = pqp.tile([128, 4, PP], F16, name=f"ptq_{kq}", tag="ptq")
                    p8 = pp_.tile([128, 4, PP], F8, name=f"pt8_{kq}", tag=f"pt8_{kq}")
                    for ks in range(4):
                        nc.scalar.dma_start(
                            out=ptq_t[:, ks, :],
                            in_=ptq_d[kq * 128:(kq + 1) * 128,
                                      ks * PP:(ks + 1) * PP])
                        nc.vector.tensor_scalar_mul(p8[:, ks, :],
                                                    ptq_t[:, ks, :], SF_P)
                    pt8_sb.append(p8)
                    tf_t = wp.tile([128, 4, 2 * E], F16, name=f"tf_{kq}", tag=f"w{kq}")
                    eng = nc.sync if kq % 2 == 0 else nc.gpsimd
                    for ks in range(4):
                        eng.dma_start(
                            out=tf_t[:, ks, :],
                            in_=tf_d[kq * 128:(kq + 1) * 128,
                                     ks * 2 * E:(ks + 1) * 2 * E])
                    for ks in range(4):
                        k = kq * 4 + ks
                        for wi in (0, 1):
                            for e in range(ET):
                                nc.tensor.matmul(
                                    ps_attn[(wi, e)][:, :],
                                    tf_t[:, ks, wi * E + e * 128: wi * E + (e + 1) * 128],
                                    ptq_t[:, ks, :],
                                    start=(k == 0), stop=(k == KT - 1))
                proj_sb = {}
                for e in range(ET):
                    for wi in (0, 1):
                        sb = pp_.tile([128, PP], F16, name=f"proj_{wi}_{e}",
                                      tag=f"proj_{wi}_{e}")
                        if bias_sb[wi]:
                            nc.scalar.activation(sb[:, :], ps_attn[(wi, e)][:, :],
                                                 mybir.ActivationFunctionType.Identity,
                                                 bias=bias_sb[wi][e][:, :], scale=1.0)
                        elif wi == 0:
                            nc.scalar.copy(sb[:, :], ps_attn[(wi, e)][:, :])
                        else:
                            nc.vector.tensor_copy(sb[:, :], ps_attn[(wi, e)][:, :])
                        proj_sb[(wi, e)] = sb

            # ---- scores + softmax per (sample, p-chunk); fold fp8 unscale ----
            wgt = {}
            with tc.tile_pool(name="psB", bufs=1, space="PSUM") as psB:
                sps_t = {}
                for (s, pc) in GRP:
                    sps_t[(s, pc)] = psB.tile([128, P], F32,
                                              name=f"ps_sc_{s}_{pc}", tag="sc", bufs=4)
                for e in range(ET):
                    for (s, pc) in GRP:
                        col = s * P + pc * 128
                        nc.tensor.matmul(sps_t[(s, pc)][:, :],
                                         proj_sb[(0, e)][:, col:col + 128],
                                         proj_sb[(1, e)][:, s * P:(s + 1) * P],
                                         start=(e == 0), stop=(e == ET - 1))
                for (s, pc) in GRP:
                    sps = sps_t[(s, pc)]
                    mx = smp.tile([128, 1], F32, name=f"mx_{s}_{pc}", tag="mx")
                    nc.vector.tensor_reduce(out=mx[:, :], in_=sps[:, :],
                                            axis=mybir.AxisListType.X, op=mybir.AluOpType.max)
                    ngm = smp.tile([128, 1], F32, name=f"ngm_{s}_{pc}", tag="ngm")
                    nc.vector.tensor_scalar_mul(ngm[:, :], mx[:, :], -1.0)
                    ex = smp.tile([128, P], F32, name=f"ex_{s}_{pc}", tag="ex")
                    ssum = smp.tile([128, 1], F32, name=f"ssum_{s}_{pc}", tag="ssum")
                    nc.scalar.activation(ex[:, :], sps[:, :], mybir.ActivationFunctionType.Exp,
                                         bias=ngm[:, :], scale=1.0, accum_out=ssum[:, :])
                    ssc = smp.tile([128, 1], F32, name=f"ssc_{s}_{pc}", tag="ssc")
                    nc.vector.tensor_scalar_mul(ssc[:, :], ssum[:, :], float(SF_P * SF_G))
                    rec = smp.tile([128, 1], F32, name=f"rec_{s}_{pc}", tag="rec")
                    nc.vector.reciprocal(rec[:, :], ssc[:, :])
                    wt_ = pp_.tile([128, P], F32, name=f"wgt_{s}_{pc}", tag=f"wgt_{s}_{pc}")
                    nc.vector.tensor_scalar_mul(wt_[:, :], ex[:, :], rec[:, :])
                    wgt[(s, pc)] = wt_

            # ---- g projection (fp8 DoubleRow) + gating + residual ----
            GW_ENG = [nc.sync, nc.gpsimd, nc.sync, nc.gpsimd,
                      nc.sync, nc.gpsimd, nc.sync, nc.gpsimd]
            OUT_ENG = [nc.scalar, nc.sync, nc.gpsimd, None]
            with tc.tile_pool(name="psC", bufs=1, space="PSUM") as psC:
                for dp in range(DP):
                    en16 = {}
                    pn16 = {}
                    for gi, (s, pc) in enumerate(GRP):
                        en16[gi] = ep.tile([128, 1024], F16, name=f"en_{dp}_{gi}", tag="en")
                        pn16[gi] = pnp.tile([128, 1024], F16, name=f"pn_{dp}_{gi}", tag="pn")
                        row = s * P + pc * 128
                        nc.scalar.dma_start(
                            out=pn16[gi][:, :],
                            in_=pnat_d[row:row + 128, dp * 1024:(dp + 1) * 1024])

                    def gate(dch, gi, s, pc, g_ps, dp=dp, en16=en16, pn16=pn16):
                        if gb_sb is not None:
                            nc.vector.tensor_add(
                                g_ps[:, :], g_ps[:, :],
                                gb_sb[0:1, dch * 512:(dch + 1) * 512]
                                .partition_broadcast(128))
                        base = (dch % 2) * 512
                        row = s * P + pc * 128
                        final = (dp == DP - 1 and gi == len(GRP) - 1)
                        if final and dch % 2 == 1:
                            # last tile of the kernel: finest-grain gating so
                            # each 256-col chunk's write starts immediately
                            for q in range(2):
                                a = base + q * 256
                                nc.vector.tensor_mul(en16[gi][:, a:a + 256],
                                                     g_ps[:, q * 256:(q + 1) * 256],
                                                     wgt[(s, pc)][:, :])
                                nc.vector.tensor_add(en16[gi][:, a:a + 256],
                                                     en16[gi][:, a:a + 256],
                                                     pn16[gi][:, a:a + 256])
                                eng = nc.sync if q == 0 else nc.gpsimd
                                eng.dma_start(
                                    out=out_d[row:row + 128,
                                              dp * 1024 + a:dp * 1024 + a + 256],
                                    in_=en16[gi][:, a:a + 256])
                            return
                        nc.vector.tensor_mul(en16[gi][:, base:base + 256],
                                             g_ps[:, 0:256], wgt[(s, pc)][:, :])
                        nc.vector.tensor_mul(en16[gi][:, base + 256:base + 512],
                                             g_ps[:, 256:512], wgt[(s, pc)][:, :])
                        nc.vector.tensor_add(en16[gi][:, base:base + 512],
                                             en16[gi][:, base:base + 512],
                                             pn16[gi][:, base:base + 512])
                        if final and dch % 2 == 0:
                            # write the first half of the final tile early
                            nc.scalar.dma_start(
                                out=out_d[row:row + 128, dp * 1024:dp * 1024 + 512],
                                in_=en16[gi][:, 0:512])
                        elif dch % 2 == 1:
                            eng = OUT_ENG[gi] if dp == DP - 1 else nc.scalar
                            eng.dma_start(
                                out=out_d[row:row + 128, dp * 1024:(dp + 1) * 1024],
                                in_=en16[gi][:, :])

                    # one 512KB descriptor per (dp, kq) covers both sub-rounds:
                    # tile free layout (sub, ks, n); rings alternate by kq
                    gts = []
                    for kq in range(KT4):
                        gt = gp.tile([128, 8, 512], F8, name=f"gt_{dp}_{kq}", tag="gt")
                        eng = nc.sync if kq % 2 == 0 else nc.gpsimd
                        eng.dma_start(
                            out=gt[:, :, :],
                            in_=gw8_d[kq * 128:(kq + 1) * 128,
                                      dp * 4096:(dp + 1) * 4096])
                        gts.append(gt)
                    for sub in range(2):
                        dch = dp * 2 + sub
                        last = (dp == DP - 1 and sub == 1)
                        if not last:
                            gps = {}
                            for gi, (s, pc) in enumerate(GRP):
                                gps[gi] = psC.tile([128, 512], F32,
                                                   name=f"ps_g_{dch}_{gi}", tag="g", bufs=6)
                            for kq in range(KT4):
                                for gi, (s, pc) in enumerate(GRP):
                                    col = s * P + pc * 128
                                    for pr in range(2):
                                        nc.tensor.matmul(
                                            gps[gi][:, :],
                                            pt8_sb[kq][:, 2 * pr:2 * pr + 2, col:col + 128],
                                            gts[kq][:, sub * 4 + 2 * pr:sub * 4 + 2 * pr + 2, :],
                                            start=(kq == 0 and pr == 0),
                                            stop=(kq == KT4 - 1 and pr == 1),
                                            perf_mode=DR)
                            for gi, (s, pc) in enumerate(GRP):
                                gate(dch, gi, s, pc, gps[gi])
                        else:
                            # final round k-inner per group: gating/writes of
                            # earlier groups overlap the remaining matmuls.
                            # The very last group accumulates in two 256-col
                            # chains so chunk A's gate+write overlaps chunk
                            # B's matmuls (nothing can hide the tail after B).
                            for gi, (s, pc) in enumerate(GRP):
                                col = s * P + pc * 128
                                g_ps = psC.tile([128, 512], F32,
                                                name=f"ps_g_{dch}_{gi}", tag="g", bufs=6)
                                if gi < len(GRP) - 1:
                                    for kq in range(KT4):
                                        for pr in range(2):
                                            nc.tensor.matmul(
                                                g_ps[:, :],
                                                pt8_sb[kq][:, 2 * pr:2 * pr + 2, col:col + 128],
                                                gts[kq][:, sub * 4 + 2 * pr:sub * 4 + 2 * pr + 2, :],
                                                start=(kq == 0 and pr == 0),
                                                stop=(kq == KT4 - 1 and pr == 1),
                                                perf_mode=DR)
                                    gate(dch, gi, s, pc, g_ps)
                                    continue
                                row = s * P + pc * 128
                                base = (dch % 2) * 512
                                for q in range(2):
                                    for kq in range(KT4):
                                        for pr in range(2):
                                            nc.tensor.matmul(
                                                g_ps[:, q * 256:(q + 1) * 256],
                                                pt8_sb[kq][:, 2 * pr:2 * pr + 2, col:col + 128],
                                                gts[kq][:, sub * 4 + 2 * pr:sub * 4 + 2 * pr + 2,
                                                        q * 256:(q + 1) * 256],
                                                start=(kq == 0 and pr == 0),
                                                stop=(kq == KT4 - 1 and pr == 1),
                                                perf_mode=DR,
                                                skip_group_check=True)
                                    a = base + q * 256
                                    nc.vector.tensor_mul(en16[gi][:, a:a + 256],
                                                         g_ps[:, q * 256:(q + 1) * 256],
                                                         wgt[(s, pc)][:, :])
                                    nc.vector.tensor_add(en16[gi][:, a:a + 256],
                                                         en16[gi][:, a:a + 256],
                                                         pn16[gi][:, a:a + 256])
                                    eng = nc.sync if q == 0 else nc.gpsimd
                                    eng.dma_start(
                                        out=out_d[row:row + 128,
                                                  dp * 1024 + a:dp * 1024 + a + 256],
                                        in_=en16[gi][:, a:a + 256])

    nc.compile()
    _built[key] = nc
    return nc


def kernel(**inputs):
    global LAST_RESULTS
    x = np.ascontiguousarray(inputs["x"], dtype=np.float32)
    tw = np.asarray(inputs["theta_w"], dtype=np.float32)
    fw = np.asarray(inputs["f_w"], dtype=np.float32)
    gw = np.asarray(inputs["g_w"], dtype=np.float32)
    tb = np.asarray(inputs["theta_b"], dtype=np.float32)
    fb = np.asarray(inputs["f_b"], dtype=np.float32)
    gb = np.asarray(inputs["g_b"], dtype=np.float32)
    scale = float(np.asarray(inputs["scale"], dtype=np.float32).reshape(-1)[0])

    with_tb = bool(np.any(tb))
    with_fb = bool(np.any(fb))
    with_gb = bool(np.any(gb))
    nc = _build(with_tb, with_fb, with_gb)

    F8NP = mybir.dt.np(F8)
    # patchify: [B,C,H,W] -> [B,P,D] with D ordered (c, u, v)
    p = x.reshape(B, C, NPS, PH, NPS, PW).transpose(0, 2, 4, 1, 3, 5).reshape(B, P, D)
    tf16 = np.concatenate([tw, fw], axis=1).astype(np.float16)
    # k-quad interleave: row kq*128+part, col ks*1024 + wcol
    tf16 = np.ascontiguousarray(
        tf16.reshape(KT4, 4, 128, 2 * E).transpose(0, 2, 1, 3)
            .reshape(KT4 * 128, 4 * 2 * E))
    # gw8: [kq, ks, part, dch, n] -> [kq, part, dch, ks, n]
    gq = np.clip(gw * (scale * SF_G), -240.0, 240.0).astype(F8NP)
    gw8 = np.ascontiguousarray(
        gq.reshape(KT4, 4, 128, DCH, 512).transpose(0, 2, 3, 1, 4)
          .reshape(KT4 * 128, DCH * 2048))
    in_maps = []
    for ci in range(NCORES):
        p2 = p[ci * SPC:(ci + 1) * SPC]                      # [SPC, P, D]
        pnat = p2.reshape(PP, D).astype(np.float16)
        pT = p2.transpose(2, 0, 1).reshape(D, PP)
        ptq = np.ascontiguousarray(
            pT.reshape(KT4, 4, 128, PP).transpose(0, 2, 1, 3)
              .reshape(KT4 * 128, 4 * PP)).astype(np.float16)
        m = {"ptq": ptq, "pnat": pnat, "tf": tf16, "gw8": gw8}
        if with_tb:
            m["tb"] = np.ascontiguousarray(tb.reshape(E, 1))
        if with_fb:
            m["fb"] = np.ascontiguousarray(fb.reshape(E, 1))
        if with_gb:
            m["gb"] = np.ascontiguousarray((gb * (scale * SF_P * SF_G)).reshape(1, D))
        in_maps.append(m)

    res = run_bass_kernel_spmd(nc, in_maps, core_ids=list(range(NCORES)))
    LAST_RESULTS = res
    o = np.concatenate([np.asarray(res.results[ci]["out"], dtype=np.float32)
                        .reshape(SPC, P, D)
                        for ci in range(NCORES)], axis=0)     # [B, P, D]
    img = (o.reshape(B, NPS, NPS, C, PH, PW)
            .transpose(0, 3, 1, 4, 2, 5)
            .reshape(B, C, H, W))
    return np.ascontiguousarray(img, dtype=np.float32)


# revision 39
# speedup vs baseline: 1.1771x; 1.1771x over previous
"""Trainium2 Bass kernel for nn_LocalEnhancementModule (8-core SPMD, data-parallel over batch).

Per-sample computation (B=16, P=256 patches, D=4096, E=512):
    p      = patchify(x)                       [P, D]
    theta  = p @ theta_w + theta_b             [P, E]
    f      = p @ f_w + f_b                     [P, E]
    wgt    = softmax(theta @ f.T, axis=-1)     [P, P]
    g      = p @ g_w + g_b                     [P, D]
    out    = unpatchify(wgt[:,None,:] * g.reshape(P,C,P)) * scale + x

Sharding: 2 samples per core (PP=512 patch rows).

Precision/schedule: theta/f run in fp16 (the softmax scores need ~fp16
accuracy; measured fp8 scores push rel-err past tolerance). The dominant
g projection runs in fp8-e4m3 with MatmulPerfMode.DoubleRow (2 fp8
weights per PE cell, 256-deep contraction per matmul, ~2x throughput).
p is cast fp16->fp8 on-device (x16 scaling), g_w is quantized host-side
(x512 scaling, clipped to +-240); the 1/8192 unfold is folded into the
softmax reciprocal. Residual and output ride fp16 DMA (x dominates the
output norm; fp16 rounding is ~2e-4 rel). PSUM accumulates fp32
throughout.

DMA: k-quad interleaved layouts give 2-4KB contiguous rows per transfer.
Traffic is split over three rings: sync (tf even-k, gw8 even rounds),
gpsimd (tf odd-k, gw8 odd rounds), scalar (p tiles, residual, output).
"""

import sys
import numpy as np

try:
    import concourse.bacc as bacc
except ImportError:  # pragma: no cover
    for _p in ("/opt/trn_rl_repo", "/root/.axon_site/_ro/trn_rl_repo"):
        if _p not in sys.path:
            sys.path.append(_p)
    import concourse.bacc as bacc
import concourse.mybir as mybir
import concourse.tile as tile
from concourse.bass_utils import run_bass_kernel_spmd

NCORES = 8
B, C, H, W = 16, 16, 256, 256
NPS, PH, PW = 16, 16, 16
P = NPS * NPS            # 256 patches
D = C * PH * PW          # 4096
E = 512
SPC = B // NCORES        # 2 samples per core
PP = SPC * P             # 512 patch rows per core
KT = D // 128            # 32 contraction tiles of 128
KT4 = D // 512           # 8 k-quad tiles (4 x 128)
ET = E // 128            # 4 embedding chunks
DCH = D // 512           # 8 column rounds for g
DP = DCH // 2            # 4 column-pair rounds (1024-wide output writes)
GRP = [(s, pc) for s in range(SPC) for pc in range(2)]

SF_P = 16.0              # fp8 scale for p
SF_G = 512.0             # fp8 scale for g_w
UNSCALE = 1.0 / (SF_P * SF_G)

F32 = mybir.dt.float32
F16 = mybir.dt.float16
F8 = mybir.dt.float8e4
DR = mybir.MatmulPerfMode.DoubleRow

_built = {}
LAST_RESULTS = None  # stashed BassKernelResults for test harness introspection


def _build(with_tb, with_fb, with_gb):
    key = (with_tb, with_fb, with_gb)
    if key in _built:
        return _built[key]

    nc = bacc.Bacc("TRN2", num_devices=NCORES, debug=False)
    # ptq: pT fp16, k-quad interleaved: row kq*128+part, col ks*PP+pp,
    #      element = pT[k=kq*512+ks*128+part, pp]
    ptq_d = nc.dram_tensor("ptq", [KT4 * 128, 4 * PP], F16, kind="ExternalInput").ap()
    pnat_d = nc.dram_tensor("pnat", [PP, D], F16, kind="ExternalInput").ap()
    # tf: concat(theta_w | f_w) columns, k-quad interleaved like ptq:
    # row kq*128+part, col ks*1024 + wcol; element = tf[k=kq*512+ks*128+part, wcol]
    tf_d = nc.dram_tensor("tf", [KT4 * 128, 4 * 2 * E], F16, kind="ExternalInput").ap()
    # gw8: fp8 g_w * scale * SF_G; row kq*128+part, col dch*2048+ks*512+n,
    #      element = gw[k=kq*512+ks*128+part, d=dch*512+n]
    gw8_d = nc.dram_tensor("gw8", [KT4 * 128, DCH * 2048], F8, kind="ExternalInput").ap()
    tb_d = nc.dram_tensor("tb", [E, 1], F32, kind="ExternalInput").ap() if with_tb else None
    fb_d = nc.dram_tensor("fb", [E, 1], F32, kind="ExternalInput").ap() if with_fb else None
    gb_d = nc.dram_tensor("gb", [1, D], F32, kind="ExternalInput").ap() if with_gb else None
    out_d = nc.dram_tensor("out", [PP, D], F16, kind="ExternalOutput").ap()

    with tile.TileContext(nc) as tc:
        with tc.tile_pool(name="persist", bufs=1) as pp_, \
             tc.tile_pool(name="ptstream", bufs=8) as pqp, \
             tc.tile_pool(name="wstream", bufs=1) as wp, \
             tc.tile_pool(name="gstream", bufs=10) as gp, \
             tc.tile_pool(name="pnstream", bufs=6) as pnp, \
             tc.tile_pool(name="enh", bufs=6) as ep, \
             tc.tile_pool(name="sm", bufs=4) as smp:

            # PE warmup: ~48 trivial matmuls during the initial DMA wait so
            # the HAM clock-gate reaches K=8/8 before the first real matmul
            with tc.tile_pool(name="warm", bufs=1) as wmp, \
                 tc.tile_pool(name="psW", bufs=1, space="PSUM") as psW:
                wt_warm = wmp.tile([128, 128], F16, name="wt_warm", tag="warm")
                nc.gpsimd.memset(wt_warm[:, :], 0.0)
                ps_warm = psW.tile([128, 128], F32, name="ps_warm", tag="psw")
                for _ in range(36):
                    nc.tensor.matmul(ps_warm[:, :], wt_warm[:, :], wt_warm[:, :],
                                     start=True, stop=True)

            bias_sb = {0: [], 1: []}
            for wi, bd in ((0, tb_d), (1, fb_d)):
                if bd is None:
                    continue
                for e in range(ET):
                    bt = pp_.tile([128, 1], F32, name=f"bias_{wi}_{e}", tag=f"bias_{wi}_{e}")
                    nc.scalar.dma_start(out=bt[:, :], in_=bd[e * 128:(e + 1) * 128, :])
                    bias_sb[wi].append(bt)
            gb_sb = None
            if gb_d is not None:
                gb_sb = pp_.tile([1, D], F32, name="gb_sb", tag="gb_sb")
                nc.scalar.dma_start(out=gb_sb[:, :], in_=gb_d[:, :])

            # ---- theta / f projections (fp16), k-outer into 8 PSUM banks ----
            pt8_sb = []
            with tc.tile_pool(name="psA", bufs=1, space="PSUM") as psA:
                ps_attn = {}
                for wi in (0, 1):
                    for e in range(ET):
                        ps_attn[(wi, e)] = psA.tile([128, PP], F32,
                                                    name=f"ps_attn_{wi}_{e}",
                                                    tag=f"attn_{wi}_{e}")
                for kq in range(KT4):
                    # per-ks descriptors + persistent tiles: fine-grained
                    # arrival keeps PE catch-up stalls under the 3.4us HAM
                    # re-throttle window, and no buffer-reuse WAR waits (a
                    # waiting DMA trigger head-of-line blocks its engine)
                    ptq_t = pqp.tile([128, 4, PP], F16, name=f"ptq_{kq}", tag="ptq")
                    p8 = pp_.tile([128, 4, PP], F8, name=f"pt8_{kq}", tag=f"pt8_{kq}")
                    for ks in range(4):
                        nc.scalar.dma_start(
                            out=ptq_t[:, ks, :],
                            in_=ptq_d[kq * 128:(kq + 1) * 128,
                                      ks * PP:(ks + 1) * PP])
                        nc.vector.tensor_scalar_mul(p8[:, ks, :],
                                                    ptq_t[:, ks, :], SF_P)
                    pt8_sb.append(p8)
                    tf_t = wp.tile([128, 4, 2 * E], F16, name=f"tf_{kq}", tag=f"w{kq}")
                    eng = nc.sync if kq % 2 == 0 else nc.gpsimd
                    for ks in range(4):
                        eng.dma_start(
                            out=tf_t[:, ks, :],
                            in_=tf_d[kq * 128:(kq + 1) * 128,
                                     ks * 2 * E:(ks + 1) * 2 * E])
                    for ks in range(4):
                        k = kq * 4 + ks
                        for wi in (0, 1):
                            for e in range(ET):
                                nc.tensor.matmul(
                                    ps_attn[(wi, e)][:, :],
                                    tf_t[:, ks, wi * E + e * 128: wi * E + (e + 1) * 128],
                                    ptq_t[:, ks, :],
                                    start=(k == 0), stop=(k == KT - 1))
                proj_sb = {}
                for e in range(ET):
                    for wi in (0, 1):
                        sb = pp_.tile([128, PP], F16, name=f"proj_{wi}_{e}",
                                      tag=f"proj_{wi}_{e}")
                        if bias_sb[wi]:
                            nc.scalar.activation(sb[:, :], ps_attn[(wi, e)][:, :],
                                                 mybir.ActivationFunctionType.Identity,
                                                 bias=bias_sb[wi][e][:, :], scale=1.0)
                        elif wi == 0:
                            nc.scalar.copy(sb[:, :], ps_attn[(wi, e)][:, :])
                        else:
                            nc.vector.tensor_copy(sb[:, :], ps_attn[(wi, e)][:, :])
                        proj_sb[(wi, e)] = sb

            # ---- scores + softmax per (sample, p-chunk); fold fp8 unscale ----
            wgt = {}
            with tc.tile_pool(name="psB", bufs=1, space="PSUM") as psB:
                sps_t = {}
                for (s, pc) in GRP:
                    sps_t[(s, pc)] = psB.tile([128, P], F32,
                                              name=f"ps_sc_{s}_{pc}", tag="sc", bufs=4)
                for e in range(ET):
                    for (s, pc) in GRP:
                        col = s * P + pc * 128
                        nc.tensor.matmul(sps_t[(s, pc)][:, :],
                                         proj_sb[(0, e)][:, col:col + 128],
                                         proj_sb[(1, e)][:, s * P:(s + 1) * P],
                                         start=(e == 0), stop=(e == ET - 1))
                for (s, pc) in GRP:
                    sps = sps_t[(s, pc)]
                    mx = smp.tile([128, 1], F32, name=f"mx_{s}_{pc}", tag="mx")
                    nc.vector.tensor_reduce(out=mx[:, :], in_=sps[:, :],
                                            axis=mybir.AxisListType.X, op=mybir.AluOpType.max)
                    ngm = smp.tile([128, 1], F32, name=f"ngm_{s}_{pc}", tag="ngm")
                    nc.vector.tensor_scalar_mul(ngm[:, :], mx[:, :], -1.0)
                    ex = smp.tile([128, P], F32, name=f"ex_{s}_{pc}", tag="ex")
                    ssum = smp.tile([128, 1], F32, name=f"ssum_{s}_{pc}", tag="ssum")
                    nc.scalar.activation(ex[:, :], sps[:, :], mybir.ActivationFunctionType.Exp,
                                         bias=ngm[:, :], scale=1.0, accum_out=ssum[:, :])
                    ssc = smp.tile([128, 1], F32, name=f"ssc_{s}_{pc}", tag="ssc")
                    nc.vector.tensor_scalar_mul(ssc[:, :], ssum[:, :], float(SF_P * SF_G))
                    rec = smp.tile([128, 1], F32, name=f"rec_{s}_{pc}", tag="rec")
                    nc.vector.reciprocal(rec[:, :], ssc[:, :])
                    wt_ = pp_.tile([128, P], F32, name=f"wgt_{s}_{pc}", tag=f"wgt_{s}_{pc}")
                    nc.vector.tensor_scalar_mul(wt_[:, :], ex[:, :], rec[:, :])
                    wgt[(s, pc)] = wt_

            # ---- g projection (fp8 DoubleRow) + gating + residual ----
            GW_ENG = [nc.sync, nc.gpsimd, nc.sync, nc.gpsimd,
                      nc.sync, nc.gpsimd, nc.sync, nc.gpsimd]
            OUT_ENG = [nc.scalar, nc.sync, nc.gpsimd, None]
            with tc.tile_pool(name="psC", bufs=1, space="PSUM") as psC:
                for dp in range(DP):
                    en16 = {}
                    pn16 = {}
                    for gi, (s, pc) in enumerate(GRP):
                        en16[gi] = ep.tile([128, 1024], F16, name=f"en_{dp}_{gi}", tag="en")
                        pn16[gi] = pnp.tile([128, 1024], F16, name=f"pn_{dp}_{gi}", tag="pn")
                        row = s * P + pc * 128
                        nc.scalar.dma_start(
                            out=pn16[gi][:, :],
                            in_=pnat_d[row:row + 128, dp * 1024:(dp + 1) * 1024])

                    def gate(dch, gi, s, pc, g_ps, dp=dp, en16=en16, pn16=pn16):
                        if gb_sb is not None:
                            nc.vector.tensor_add(
                                g_ps[:, :], g_ps[:, :],
                                gb_sb[0:1, dch * 512:(dch + 1) * 512]
                                .partition_broadcast(128))
                        base = (dch % 2) * 512
                        row = s * P + pc * 128
                        final = (dp == DP - 1 and gi == len(GRP) - 1)
                        if final and dch % 2 == 1:
                            # last tile of the kernel: finest-grain gating so
                            # each 256-col chunk's write starts immediately
                            for q in range(2):
                                a = base + q * 256
                                nc.vector.tensor_mul(en16[gi][:, a:a + 256],
                                                     g_ps[:, q * 256:(q + 1) * 256],
                                                     wgt[(s, pc)][:, :])
                                nc.vector.tensor_add(en16[gi][:, a:a + 256],
                                                     en16[gi][:, a:a + 256],
                                                     pn16[gi][:, a:a + 256])
                                eng = nc.sync if q == 0 else nc.gpsimd
                                eng.dma_start(
                                    out=out_d[row:row + 128,
                                              dp * 1024 + a:dp * 1024 + a + 256],
                                    in_=en16[gi][:, a:a + 256])
                            return
                        nc.vector.tensor_mul(en16[gi][:, base:base + 256],
                                             g_ps[:, 0:256], wgt[(s, pc)][:, :])
                        nc.vector.tensor_mul(en16[gi][:, base + 256:base + 512],
                                             g_ps[:, 256:512], wgt[(s, pc)][:, :])
                        nc.vector.tensor_add(en16[gi][:, base:base + 512],
                                             en16[gi][:, base:base + 512],
                                             pn16[gi][:, base:base + 512])
                        if final and dch % 2 == 0:
                            # write the first half of the final tile early
                            nc.scalar.dma_start(
                                out=out_d[row:row + 128, dp * 1024:dp * 1024 + 512],
                                in_=en16[gi][:, 0:512])
                        elif dch % 2 == 1:
                            eng = OUT_ENG[gi] if dp == DP - 1 else nc.scalar
                            eng.dma_start(
                                out=out_d[row:row + 128, dp * 1024:(dp + 1) * 1024],
                                in_=en16[gi][:, :])

                    # one 512KB descriptor per (dp, kq) covers both sub-rounds:
                    # tile free layout (sub, ks, n); rings alternate by kq
                    gts = []
                    for kq in range(KT4):
                        gt = gp.tile([128, 8, 512], F8, name=f"gt_{dp}_{kq}", tag="gt")
                        eng = nc.sync if kq % 2 == 0 else nc.gpsimd
                        eng.dma_start(
                            out=gt[:, :, :],
                            in_=gw8_d[kq * 128:(kq + 1) * 128,
                                      dp * 4096:(dp + 1) * 4096])
                        gts.append(gt)
                    for sub in range(2):
                        dch = dp * 2 + sub
                        last = (dp == DP - 1 and sub == 1)
                        if not last:
                            gps = {}
                            for gi, (s, pc) in enumerate(GRP):
                                gps[gi] = psC.tile([128, 512], F32,
                                                   name=f"ps_g_{dch}_{gi}", tag="g", bufs=6)
                            for kq in range(KT4):
                                for gi, (s, pc) in enumerate(GRP):
                                    col = s * P + pc * 128
                                    for pr in range(2):
                                        nc.tensor.matmul(
                                            gps[gi][:, :],
                                            pt8_sb[kq][:, 2 * pr:2 * pr + 2, col:col + 128],
                                            gts[kq][:, sub * 4 + 2 * pr:sub * 4 + 2 * pr + 2, :],
                                            start=(kq == 0 and pr == 0),
                                            stop=(kq == KT4 - 1 and pr == 1),
                                            perf_mode=DR)
                            for gi, (s, pc) in enumerate(GRP):
                                gate(dch, gi, s, pc, gps[gi])
                        else:
                            # final round k-inner per group: gating/writes of
                            # earlier groups overlap the remaining matmuls
                            for gi, (s, pc) in enumerate(GRP):
                                col = s * P + pc * 128
                                g_ps = psC.tile([128, 512], F32,
                                                name=f"ps_g_{dch}_{gi}", tag="g", bufs=6)
                                for kq in range(KT4):
                                    for pr in range(2):
                                        nc.tensor.matmul(
                                            g_ps[:, :],
                                            pt8_sb[kq][:, 2 * pr:2 * pr + 2, col:col + 128],
                                            gts[kq][:, sub * 4 + 2 * pr:sub * 4 + 2 * pr + 2, :],
                                            start=(kq == 0 and pr == 0),
                                            stop=(kq == KT4 - 1 and pr == 1),
                                            perf_mode=DR)
                                gate(dch, gi, s, pc, g_ps)

    nc.compile()
    _built[key] = nc
    return nc


def kernel(**inputs):
    global LAST_RESULTS
    x = np.ascontiguousarray(inputs["x"], dtype=np.float32)
    tw = np.asarray(inputs["theta_w"], dtype=np.float32)
    fw = np.asarray(inputs["f_w"], dtype=np.float32)
    gw = np.asarray(inputs["g_w"], dtype=np.float32)
    tb = np.asarray(inputs["theta_b"], dtype=np.float32)
    fb = np.asarray(inputs["f_b"], dtype=np.float32)
    gb = np.asarray(inputs["g_b"], dtype=np.float32)
    scale = float(np.asarray(inputs["scale"], dtype=np.float32).reshape(-1)[0])

    with_tb = bool(np.any(tb))
    with_fb = bool(np.any(fb))
    with_gb = bool(np.any(gb))
    nc = _build(with_tb, with_fb, with_gb)

    F8NP = mybir.dt.np(F8)
    # patchify: [B,C,H,W] -> [B,P,D] with D ordered (c, u, v)
    p = x.reshape(B, C, NPS, PH, NPS, PW).transpose(0, 2, 4, 1, 3, 5).reshape(B, P, D)
    tf16 = np.concatenate([tw, fw], axis=1).astype(np.float16)
    # k-quad interleave: row kq*128+part, col ks*1024 + wcol
    tf16 = np.ascontiguousarray(
        tf16.reshape(KT4, 4, 128, 2 * E).transpose(0, 2, 1, 3)
            .reshape(KT4 * 128, 4 * 2 * E))
    # gw8: [kq, ks, part, dch, n] -> [kq, part, dch, ks, n]
    gq = np.clip(gw * (scale * SF_G), -240.0, 240.0).astype(F8NP)
    gw8 = np.ascontiguousarray(
        gq.reshape(KT4, 4, 128, DCH, 512).transpose(0, 2, 3, 1, 4)
          .reshape(KT4 * 128, DCH * 2048))
    in_maps = []
    for ci in range(NCORES):
        p2 = p[ci * SPC:(ci + 1) * SPC]                      # [SPC, P, D]
        pnat = p2.reshape(PP, D).astype(np.float16)
        pT = p2.transpose(2, 0, 1).reshape(D, PP)
        ptq = np.ascontiguousarray(
            pT.reshape(KT4, 4, 128, PP).transpose(0, 2, 1, 3)
              .reshape(KT4 * 128, 4 * PP)).astype(np.float16)
        m = {"ptq": ptq, "pnat": pnat, "tf": tf16, "gw8": gw8}
        if with_tb:
            m["tb"] = np.ascontiguousarray(tb.reshape(E, 1))
        if with_fb:
            m["fb"] = np.ascontiguousarray(fb.reshape(E, 1))
        if with_gb:
            m["gb"] = np.ascontiguousarray((gb * (scale * SF_P * SF_G)).reshape(1, D))
        in_maps.append(m)

    res = run_bass_kernel_spmd(nc, in_maps, core_ids=list(range(NCORES)))
    LAST_RESULTS = res
    o = np.concatenate([np.asarray(res.results[ci]["out"], dtype=np.float32)
                        .reshape(SPC, P, D)
                        for ci in range(NCORES)], axis=0)     # [B, P, D]
    img = (o.reshape(B, NPS, NPS, C, PH, PW)
            .transpose(0, 3, 1, 4, 2, 5)
            .reshape(B, C, H, W))
    return np.ascontiguousarray(img, dtype=np.float32)


# revision 40
# speedup vs baseline: 1.1899x; 1.0109x over previous
"""Trainium2 Bass kernel for nn_LocalEnhancementModule (8-core SPMD, data-parallel over batch).

Per-sample computation (B=16, P=256 patches, D=4096, E=512):
    p      = patchify(x)                       [P, D]
    theta  = p @ theta_w + theta_b             [P, E]
    f      = p @ f_w + f_b                     [P, E]
    wgt    = softmax(theta @ f.T, axis=-1)     [P, P]
    g      = p @ g_w + g_b                     [P, D]
    out    = unpatchify(wgt[:,None,:] * g.reshape(P,C,P)) * scale + x

Sharding: 2 samples per core (PP=512 patch rows).

Precision/schedule: theta/f run in fp16 (the softmax scores need ~fp16
accuracy; measured fp8 scores push rel-err past tolerance). The dominant
g projection runs in fp8-e4m3 with MatmulPerfMode.DoubleRow (2 fp8
weights per PE cell, 256-deep contraction per matmul, ~2x throughput).
p is cast fp16->fp8 on-device (x16 scaling), g_w is quantized host-side
(x512 scaling, clipped to +-240); the 1/8192 unfold is folded into the
softmax reciprocal. Residual and output ride fp16 DMA (x dominates the
output norm; fp16 rounding is ~2e-4 rel). PSUM accumulates fp32
throughout.

DMA: k-quad interleaved layouts give 2-4KB contiguous rows per transfer.
Traffic is split over three rings: sync (tf even-k, gw8 even rounds),
gpsimd (tf odd-k, gw8 odd rounds), scalar (p tiles, residual, output).
"""

import sys
import numpy as np

try:
    import concourse.bacc as bacc
except ImportError:  # pragma: no cover
    for _p in ("/opt/trn_rl_repo", "/root/.axon_site/_ro/trn_rl_repo"):
        if _p not in sys.path:
            sys.path.append(_p)
    import concourse.bacc as bacc
import concourse.mybir as mybir
import concourse.tile as tile
from concourse.bass_utils import run_bass_kernel_spmd

NCORES = 8
B, C, H, W = 16, 16, 256, 256
NPS, PH, PW = 16, 16, 16
P = NPS * NPS            # 256 patches
D = C * PH * PW          # 4096
E = 512
SPC = B // NCORES        # 2 samples per core
PP = SPC * P             # 512 patch rows per core
KT = D // 128            # 32 contraction tiles of 128
KT4 = D // 512           # 8 k-quad tiles (4 x 128)
ET = E // 128            # 4 embedding chunks
DCH = D // 512           # 8 column rounds for g
DP = DCH // 2            # 4 column-pair rounds (1024-wide output writes)
GRP = [(s, pc) for s in range(SPC) for pc in range(2)]

SF_P = 16.0              # fp8 scale for p
SF_G = 512.0             # fp8 scale for g_w
UNSCALE = 1.0 / (SF_P * SF_G)

F32 = mybir.dt.float32
F16 = mybir.dt.float16
F8 = mybir.dt.float8e4
DR = mybir.MatmulPerfMode.DoubleRow

_built = {}
LAST_RESULTS = None  # stashed BassKernelResults for test harness introspection


def _build(with_tb, with_fb, with_gb):
    key = (with_tb, with_fb, with_gb)
    if key in _built:
        return _built[key]

    nc = bacc.Bacc("TRN2", num_devices=NCORES, debug=False)
    # ptq: pT fp16, k-quad interleaved: row kq*128+part, col ks*PP+pp,
    #      element = pT[k=kq*512+ks*128+part, pp]
    ptq_d = nc.dram_tensor("ptq", [KT4 * 128, 4 * PP], F16, kind="ExternalInput").ap()
    pnat_d = nc.dram_tensor("pnat", [PP, D], F16, kind="ExternalInput").ap()
    # tf: concat(theta_w | f_w) columns, k-quad interleaved like ptq:
    # row kq*128+part, col ks*1024 + wcol; element = tf[k=kq*512+ks*128+part, wcol]
    tf_d = nc.dram_tensor("tf", [KT4 * 128, 4 * 2 * E], F16, kind="ExternalInput").ap()
    # gw8: fp8 g_w * scale * SF_G; row kq*128+part, col dch*2048+ks*512+n,
    #      element = gw[k=kq*512+ks*128+part, d=dch*512+n]
    gw8_d = nc.dram_tensor("gw8", [KT4 * 128, DCH * 2048], F8, kind="ExternalInput").ap()
    tb_d = nc.dram_tensor("tb", [E, 1], F32, kind="ExternalInput").ap() if with_tb else None
    fb_d = nc.dram_tensor("fb", [E, 1], F32, kind="ExternalInput").ap() if with_fb else None
    gb_d = nc.dram_tensor("gb", [1, D], F32, kind="ExternalInput").ap() if with_gb else None
    out_d = nc.dram_tensor("out", [PP, D], F16, kind="ExternalOutput").ap()

    with tile.TileContext(nc) as tc:
        with tc.tile_pool(name="persist", bufs=1) as pp_, \
             tc.tile_pool(name="ptstream", bufs=8) as pqp, \
             tc.tile_pool(name="wstream", bufs=1) as wp, \
             tc.tile_pool(name="gstream", bufs=10) as gp, \
             tc.tile_pool(name="pnstream", bufs=6) as pnp, \
             tc.tile_pool(name="enh", bufs=6) as ep, \
             tc.tile_pool(name="sm", bufs=4) as smp:

            # PE warmup: ~48 trivial matmuls during the initial DMA wait so
            # the HAM clock-gate reaches K=8/8 before the first real matmul
            with tc.tile_pool(name="warm", bufs=1) as wmp, \
                 tc.tile_pool(name="psW", bufs=1, space="PSUM") as psW:
                wt_warm = wmp.tile([128, 128], F16, name="wt_warm", tag="warm")
                nc.vector.memset(wt_warm[:, :], 0.0)
                ps_warm = psW.tile([128, 128], F32, name="ps_warm", tag="psw")
                for _ in range(36):
                    nc.tensor.matmul(ps_warm[:, :], wt_warm[:, :], wt_warm[:, :],
                                     start=True, stop=True)

            bias_sb = {0: [], 1: []}
            for wi, bd in ((0, tb_d), (1, fb_d)):
                if bd is None:
                    continue
                for e in range(ET):
                    bt = pp_.tile([128, 1], F32, name=f"bias_{wi}_{e}", tag=f"bias_{wi}_{e}")
                    nc.scalar.dma_start(out=bt[:, :], in_=bd[e * 128:(e + 1) * 128, :])
                    bias_sb[wi].append(bt)
            gb_sb = None
            if gb_d is not None:
                gb_sb = pp_.tile([1, D], F32, name="gb_sb", tag="gb_sb")
                nc.scalar.dma_start(out=gb_sb[:, :], in_=gb_d[:, :])

            # ---- theta / f projections (fp16), k-outer into 8 PSUM banks ----
            pt8_sb = []
            with tc.tile_pool(name="psA", bufs=1, space="PSUM") as psA:
                ps_attn = {}
                for wi in (0, 1):
                    for e in range(ET):
                        ps_attn[(wi, e)] = psA.tile([128, PP], F32,
                                                    name=f"ps_attn_{wi}_{e}",
                                                    tag=f"attn_{wi}_{e}")
                for kq in range(KT4):
                    # per-ks descriptors + persistent tiles: fine-grained
                    # arrival keeps PE catch-up stalls under the 3.4us HAM
                    # re-throttle window, and no buffer-reuse WAR waits (a
                    # waiting DMA trigger head-of-line blocks its engine)
                    ptq_t = pqp.tile([128, 4, PP], F16, name=f"ptq_{kq}", tag="ptq")
                    p8 = pp_.tile([128, 4, PP], F8, name=f"pt8_{kq}", tag=f"pt8_{kq}")
                    for ks in range(4):
                        nc.scalar.dma_start(
                            out=ptq_t[:, ks, :],
                            in_=ptq_d[kq * 128:(kq + 1) * 128,
                                      ks * PP:(ks + 1) * PP])
                        nc.vector.tensor_scalar_mul(p8[:, ks, :],
                                                    ptq_t[:, ks, :], SF_P)
                    pt8_sb.append(p8)
                    tf_t = wp.tile([128, 4, 2 * E], F16, name=f"tf_{kq}", tag=f"w{kq}")
                    eng = nc.sync if kq % 2 == 0 else nc.gpsimd
                    for ks in range(4):
                        eng.dma_start(
                            out=tf_t[:, ks, :],
                            in_=tf_d[kq * 128:(kq + 1) * 128,
                                     ks * 2 * E:(ks + 1) * 2 * E])
                    for ks in range(4):
                        k = kq * 4 + ks
                        for wi in (0, 1):
                            for e in range(ET):
                                nc.tensor.matmul(
                                    ps_attn[(wi, e)][:, :],
                                    tf_t[:, ks, wi * E + e * 128: wi * E + (e + 1) * 128],
                                    ptq_t[:, ks, :],
                                    start=(k == 0), stop=(k == KT - 1))
                proj_sb = {}
                for e in range(ET):
                    for wi in (0, 1):
                        sb = pp_.tile([128, PP], F16, name=f"proj_{wi}_{e}",
                                      tag=f"proj_{wi}_{e}")
                        if bias_sb[wi]:
                            nc.scalar.activation(sb[:, :], ps_attn[(wi, e)][:, :],
                                                 mybir.ActivationFunctionType.Identity,
                                                 bias=bias_sb[wi][e][:, :], scale=1.0)
                        elif wi == 0:
                            nc.scalar.copy(sb[:, :], ps_attn[(wi, e)][:, :])
                        else:
                            nc.vector.tensor_copy(sb[:, :], ps_attn[(wi, e)][:, :])
                        proj_sb[(wi, e)] = sb

            # ---- scores + softmax per (sample, p-chunk); fold fp8 unscale ----
            wgt = {}
            with tc.tile_pool(name="psB", bufs=1, space="PSUM") as psB:
                sps_t = {}
                for (s, pc) in GRP:
                    sps_t[(s, pc)] = psB.tile([128, P], F32,
                                              name=f"ps_sc_{s}_{pc}", tag="sc", bufs=4)
                for e in range(ET):
                    for (s, pc) in GRP:
                        col = s * P + pc * 128
                        nc.tensor.matmul(sps_t[(s, pc)][:, :],
                                         proj_sb[(0, e)][:, col:col + 128],
                                         proj_sb[(1, e)][:, s * P:(s + 1) * P],
                                         start=(e == 0), stop=(e == ET - 1))
                for (s, pc) in GRP:
                    sps = sps_t[(s, pc)]
                    mx = smp.tile([128, 1], F32, name=f"mx_{s}_{pc}", tag="mx")
                    nc.vector.tensor_reduce(out=mx[:, :], in_=sps[:, :],
                                            axis=mybir.AxisListType.X, op=mybir.AluOpType.max)
                    ngm = smp.tile([128, 1], F32, name=f"ngm_{s}_{pc}", tag="ngm")
                    nc.vector.tensor_scalar_mul(ngm[:, :], mx[:, :], -1.0)
                    ex = smp.tile([128, P], F32, name=f"ex_{s}_{pc}", tag="ex")
                    ssum = smp.tile([128, 1], F32, name=f"ssum_{s}_{pc}", tag="ssum")
                    nc.scalar.activation(ex[:, :], sps[:, :], mybir.ActivationFunctionType.Exp,
                                         bias=ngm[:, :], scale=1.0, accum_out=ssum[:, :])
                    ssc = smp.tile([128, 1], F32, name=f"ssc_{s}_{pc}", tag="ssc")
                    nc.vector.tensor_scalar_mul(ssc[:, :], ssum[:, :], float(SF_P * SF_G))
                    rec = smp.tile([128, 1], F32, name=f"rec_{s}_{pc}", tag="rec")
                    nc.vector.reciprocal(rec[:, :], ssc[:, :])
                    wt_ = pp_.tile([128, P], F32, name=f"wgt_{s}_{pc}", tag=f"wgt_{s}_{pc}")
                    nc.vector.tensor_scalar_mul(wt_[:, :], ex[:, :], rec[:, :])
                    wgt[(s, pc)] = wt_

            # ---- g projection (fp8 DoubleRow) + gating + residual ----
            GW_ENG = [nc.sync, nc.gpsimd, nc.sync, nc.gpsimd,
                      nc.sync, nc.gpsimd, nc.sync, nc.gpsimd]
            OUT_ENG = [nc.scalar, nc.sync, nc.gpsimd, None]
            with tc.tile_pool(name="psC", bufs=1, space="PSUM") as psC:
                for dp in range(DP):
                    en16 = {}
                    pn16 = {}
                    for gi, (s, pc) in enumerate(GRP):
                        en16[gi] = ep.tile([128, 1024], F16, name=f"en_{dp}_{gi}", tag="en")
                        pn16[gi] = pnp.tile([128, 1024], F16, name=f"pn_{dp}_{gi}", tag="pn")
                        row = s * P + pc * 128
                        nc.scalar.dma_start(
                            out=pn16[gi][:, :],
                            in_=pnat_d[row:row + 128, dp * 1024:(dp + 1) * 1024])

                    def gate(dch, gi, s, pc, g_ps, dp=dp, en16=en16, pn16=pn16):
                        if gb_sb is not None:
                            nc.vector.tensor_add(
                                g_ps[:, :], g_ps[:, :],
                                gb_sb[0:1, dch * 512:(dch + 1) * 512]
                                .partition_broadcast(128))
                        base = (dch % 2) * 512
                        row = s * P + pc * 128
                        final = (dp == DP - 1 and gi == len(GRP) - 1)
                        if final and dch % 2 == 1:
                            # last tile of the kernel: finest-grain gating so
                            # each 256-col chunk's write starts immediately
                            for q in range(2):
                                a = base + q * 256
                                nc.vector.tensor_mul(en16[gi][:, a:a + 256],
                                                     g_ps[:, q * 256:(q + 1) * 256],
                                                     wgt[(s, pc)][:, :])
                                nc.vector.tensor_add(en16[gi][:, a:a + 256],
                                                     en16[gi][:, a:a + 256],
                                                     pn16[gi][:, a:a + 256])
                                eng = nc.sync if q == 0 else nc.gpsimd
                                eng.dma_start(
                                    out=out_d[row:row + 128,
                                              dp * 1024 + a:dp * 1024 + a + 256],
                                    in_=en16[gi][:, a:a + 256])
                            return
                        nc.vector.tensor_mul(en16[gi][:, base:base + 256],
                                             g_ps[:, 0:256], wgt[(s, pc)][:, :])
                        nc.vector.tensor_mul(en16[gi][:, base + 256:base + 512],
                                             g_ps[:, 256:512], wgt[(s, pc)][:, :])
                        nc.vector.tensor_add(en16[gi][:, base:base + 512],
                                             en16[gi][:, base:base + 512],
                                             pn16[gi][:, base:base + 512])
                        if final and dch % 2 == 0:
                            # write the first half of the final tile early
                            nc.scalar.dma_start(
                                out=out_d[row:row + 128, dp * 1024:dp * 1024 + 512],
                                in_=en16[gi][:, 0:512])
                        elif dch % 2 == 1:
                            eng = OUT_ENG[gi] if dp == DP - 1 else nc.scalar
                            eng.dma_start(
                                out=out_d[row:row + 128, dp * 1024:(dp + 1) * 1024],
                                in_=en16[gi][:, :])

                    # one 512KB descriptor per (dp, kq) covers both sub-rounds:
                    # tile free layout (sub, ks, n); rings alternate by kq
                    gts = []
                    for kq in range(KT4):
                        gt = gp.tile([128, 8, 512], F8, name=f"gt_{dp}_{kq}", tag="gt")
                        eng = nc.sync if kq % 2 == 0 else nc.gpsimd
                        eng.dma_start(
                            out=gt[:, :, :],
                            in_=gw8_d[kq * 128:(kq + 1) * 128,
                                      dp * 4096:(dp + 1) * 4096])
                        gts.append(gt)
                    for sub in range(2):
                        dch = dp * 2 + sub
                        last = (dp == DP - 1 and sub == 1)
                        if not last:
                            gps = {}
                            for gi, (s, pc) in enumerate(GRP):
                                gps[gi] = psC.tile([128, 512], F32,
                                                   name=f"ps_g_{dch}_{gi}", tag="g", bufs=8)
                            for kq in range(KT4):
                                for gi, (s, pc) in enumerate(GRP):
                                    col = s * P + pc * 128
                                    for pr in range(2):
                                        nc.tensor.matmul(
                                            gps[gi][:, :],
                                            pt8_sb[kq][:, 2 * pr:2 * pr + 2, col:col + 128],
                                            gts[kq][:, sub * 4 + 2 * pr:sub * 4 + 2 * pr + 2, :],
                                            start=(kq == 0 and pr == 0),
                                            stop=(kq == KT4 - 1 and pr == 1),
                                            perf_mode=DR)
                            for gi, (s, pc) in enumerate(GRP):
                                gate(dch, gi, s, pc, gps[gi])
                        else:
                            # final round k-inner per group: gating/writes of
                            # earlier groups overlap the remaining matmuls
                            for gi, (s, pc) in enumerate(GRP):
                                col = s * P + pc * 128
                                g_ps = psC.tile([128, 512], F32,
                                                name=f"ps_g_{dch}_{gi}", tag="g", bufs=8)
                                for kq in range(KT4):
                                    for pr in range(2):
                                        nc.tensor.matmul(
                                            g_ps[:, :],
                                            pt8_sb[kq][:, 2 * pr:2 * pr + 2, col:col + 128],
                                            gts[kq][:, sub * 4 + 2 * pr:sub * 4 + 2 * pr + 2, :],
                                            start=(kq == 0 and pr == 0),
                                            stop=(kq == KT4 - 1 and pr == 1),
                                            perf_mode=DR)
                                gate(dch, gi, s, pc, g_ps)

    nc.compile()
    _built[key] = nc
    return nc


def kernel(**inputs):
    global LAST_RESULTS
    x = np.ascontiguousarray(inputs["x"], dtype=np.float32)
    tw = np.asarray(inputs["theta_w"], dtype=np.float32)
    fw = np.asarray(inputs["f_w"], dtype=np.float32)
    gw = np.asarray(inputs["g_w"], dtype=np.float32)
    tb = np.asarray(inputs["theta_b"], dtype=np.float32)
    fb = np.asarray(inputs["f_b"], dtype=np.float32)
    gb = np.asarray(inputs["g_b"], dtype=np.float32)
    scale = float(np.asarray(inputs["scale"], dtype=np.float32).reshape(-1)[0])

    with_tb = bool(np.any(tb))
    with_fb = bool(np.any(fb))
    with_gb = bool(np.any(gb))
    nc = _build(with_tb, with_fb, with_gb)

    F8NP = mybir.dt.np(F8)
    # patchify: [B,C,H,W] -> [B,P,D] with D ordered (c, u, v)
    p = x.reshape(B, C, NPS, PH, NPS, PW).transpose(0, 2, 4, 1, 3, 5).reshape(B, P, D)
    tf16 = np.concatenate([tw, fw], axis=1).astype(np.float16)
    # k-quad interleave: row kq*128+part, col ks*1024 + wcol
    tf16 = np.ascontiguousarray(
        tf16.reshape(KT4, 4, 128, 2 * E).transpose(0, 2, 1, 3)
            .reshape(KT4 * 128, 4 * 2 * E))
    # gw8: [kq, ks, part, dch, n] -> [kq, part, dch, ks, n]
    gq = np.clip(gw * (scale * SF_G), -240.0, 240.0).astype(F8NP)
    gw8 = np.ascontiguousarray(
        gq.reshape(KT4, 4, 128, DCH, 512).transpose(0, 2, 3, 1, 4)
          .reshape(KT4 * 128, DCH * 2048))
    in_maps = []
    for ci in range(NCORES):
        p2 = p[ci * SPC:(ci + 1) * SPC]                      # [SPC, P, D]
        pnat = p2.reshape(PP, D).astype(np.float16)
        pT = p2.transpose(2, 0, 1).reshape(D, PP)
        ptq = np.ascontiguousarray(
            pT.reshape(KT4, 4, 128, PP).transpose(0, 2, 1, 3)
              .reshape(KT4 * 128, 4 * PP)).astype(np.float16)
        m = {"ptq": ptq, "pnat": pnat, "tf": tf16, "gw8": gw8}
        if with_tb:
            m["tb"] = np.ascontiguousarray(tb.reshape(E, 1))
        if with_fb:
            m["fb"] = np.ascontiguousarray(fb.reshape(E, 1))
        if with_gb:
            m["gb"] = np.ascontiguousarray((gb * (scale * SF_P * SF_G)).reshape(1, D))
        in_maps.append(m)

    res = run_bass_kernel_spmd(nc, in_maps, core_ids=list(range(NCORES)))
    LAST_RESULTS = res
    o = np.concatenate([np.asarray(res.results[ci]["out"], dtype=np.float32)
                        .reshape(SPC, P, D)
                        for ci in range(NCORES)], axis=0)     # [B, P, D]
    img = (o.reshape(B, NPS, NPS, C, PH, PW)
            .transpose(0, 3, 1, 4, 2, 5)
            .reshape(B, C, H, W))
    return np.ascontiguousarray(img, dtype=np.float32)


# revision 41
# speedup vs baseline: 1.1955x; 1.0046x over previous
"""Trainium2 Bass kernel for nn_LocalEnhancementModule (8-core SPMD, data-parallel over batch).

Per-sample computation (B=16, P=256 patches, D=4096, E=512):
    p      = patchify(x)                       [P, D]
    theta  = p @ theta_w + theta_b             [P, E]
    f      = p @ f_w + f_b                     [P, E]
    wgt    = softmax(theta @ f.T, axis=-1)     [P, P]
    g      = p @ g_w + g_b                     [P, D]
    out    = unpatchify(wgt[:,None,:] * g.reshape(P,C,P)) * scale + x

Sharding: 2 samples per core (PP=512 patch rows).

Precision/schedule: theta/f run in fp16 (the softmax scores need ~fp16
accuracy; measured fp8 scores push rel-err past tolerance). The dominant
g projection runs in fp8-e4m3 with MatmulPerfMode.DoubleRow (2 fp8
weights per PE cell, 256-deep contraction per matmul, ~2x throughput).
p is cast fp16->fp8 on-device (x16 scaling), g_w is quantized host-side
(x512 scaling, clipped to +-240); the 1/8192 unfold is folded into the
softmax reciprocal. Residual and output ride fp16 DMA (x dominates the
output norm; fp16 rounding is ~2e-4 rel). PSUM accumulates fp32
throughout.

DMA: k-quad interleaved layouts give 2-4KB contiguous rows per transfer.
Traffic is split over three rings: sync (tf even-k, gw8 even rounds),
gpsimd (tf odd-k, gw8 odd rounds), scalar (p tiles, residual, output).
"""

import sys
import numpy as np

try:
    import concourse.bacc as bacc
except ImportError:  # pragma: no cover
    for _p in ("/opt/trn_rl_repo", "/root/.axon_site/_ro/trn_rl_repo"):
        if _p not in sys.path:
            sys.path.append(_p)
    import concourse.bacc as bacc
import concourse.mybir as mybir
import concourse.tile as tile
from concourse.bass_utils import run_bass_kernel_spmd

NCORES = 8
B, C, H, W = 16, 16, 256, 256
NPS, PH, PW = 16, 16, 16
P = NPS * NPS            # 256 patches
D = C * PH * PW          # 4096
E = 512
SPC = B // NCORES        # 2 samples per core
PP = SPC * P             # 512 patch rows per core
KT = D // 128            # 32 contraction tiles of 128
KT4 = D // 512           # 8 k-quad tiles (4 x 128)
ET = E // 128            # 4 embedding chunks
DCH = D // 512           # 8 column rounds for g
DP = DCH // 2            # 4 column-pair rounds (1024-wide output writes)
GRP = [(s, pc) for s in range(SPC) for pc in range(2)]

SF_P = 16.0              # fp8 scale for p
SF_G = 512.0             # fp8 scale for g_w
UNSCALE = 1.0 / (SF_P * SF_G)

F32 = mybir.dt.float32
F16 = mybir.dt.float16
F8 = mybir.dt.float8e4
DR = mybir.MatmulPerfMode.DoubleRow

_built = {}
LAST_RESULTS = None  # stashed BassKernelResults for test harness introspection


def _build(with_tb, with_fb, with_gb):
    key = (with_tb, with_fb, with_gb)
    if key in _built:
        return _built[key]

    nc = bacc.Bacc("TRN2", num_devices=NCORES, debug=False)
    # ptq: pT fp16, k-quad interleaved: row kq*128+part, col ks*PP+pp,
    #      element = pT[k=kq*512+ks*128+part, pp]
    ptq_d = nc.dram_tensor("ptq", [KT4 * 128, 4 * PP], F16, kind="ExternalInput").ap()
    pnat_d = nc.dram_tensor("pnat", [PP, D], F16, kind="ExternalInput").ap()
    # tf: concat(theta_w | f_w) columns, k-quad interleaved like ptq:
    # row kq*128+part, col ks*1024 + wcol; element = tf[k=kq*512+ks*128+part, wcol]
    tf_d = nc.dram_tensor("tf", [KT4 * 128, 4 * 2 * E], F16, kind="ExternalInput").ap()
    # gw8: fp8 g_w * scale * SF_G; row kq*128+part, col dch*2048+ks*512+n,
    #      element = gw[k=kq*512+ks*128+part, d=dch*512+n]
    gw8_d = nc.dram_tensor("gw8", [KT4 * 128, DCH * 2048], F8, kind="ExternalInput").ap()
    tb_d = nc.dram_tensor("tb", [E, 1], F32, kind="ExternalInput").ap() if with_tb else None
    fb_d = nc.dram_tensor("fb", [E, 1], F32, kind="ExternalInput").ap() if with_fb else None
    gb_d = nc.dram_tensor("gb", [1, D], F32, kind="ExternalInput").ap() if with_gb else None
    out_d = nc.dram_tensor("out", [PP, D], F16, kind="ExternalOutput").ap()

    with tile.TileContext(nc) as tc:
        with tc.tile_pool(name="persist", bufs=1) as pp_, \
             tc.tile_pool(name="ptstream", bufs=8) as pqp, \
             tc.tile_pool(name="wstream", bufs=1) as wp, \
             tc.tile_pool(name="gstream", bufs=10) as gp, \
             tc.tile_pool(name="pnstream", bufs=6) as pnp, \
             tc.tile_pool(name="enh", bufs=6) as ep, \
             tc.tile_pool(name="sm", bufs=4) as smp:

            # PE warmup: ~48 trivial matmuls during the initial DMA wait so
            # the HAM clock-gate reaches K=8/8 before the first real matmul
            with tc.tile_pool(name="warm", bufs=1) as wmp, \
                 tc.tile_pool(name="psW", bufs=1, space="PSUM") as psW:
                wt_warm = wmp.tile([128, 128], F16, name="wt_warm", tag="warm")
                nc.gpsimd.memset(wt_warm[:, :], 0.0)
                ps_warm = psW.tile([128, 128], F32, name="ps_warm", tag="psw")
                for _ in range(36):
                    nc.tensor.matmul(ps_warm[:, :], wt_warm[:, :], wt_warm[:, :],
                                     start=True, stop=True)

            bias_sb = {0: [], 1: []}
            for wi, bd in ((0, tb_d), (1, fb_d)):
                if bd is None:
                    continue
                for e in range(ET):
                    bt = pp_.tile([128, 1], F32, name=f"bias_{wi}_{e}", tag=f"bias_{wi}_{e}")
                    nc.scalar.dma_start(out=bt[:, :], in_=bd[e * 128:(e + 1) * 128, :])
                    bias_sb[wi].append(bt)
            gb_sb = None
            if gb_d is not None:
                gb_sb = pp_.tile([1, D], F32, name="gb_sb", tag="gb_sb")
                nc.scalar.dma_start(out=gb_sb[:, :], in_=gb_d[:, :])

            # ---- theta / f projections (fp16), k-outer into 8 PSUM banks ----
            pt8_sb = []
            with tc.tile_pool(name="psA", bufs=1, space="PSUM") as psA:
                ps_attn = {}
                for wi in (0, 1):
                    for e in range(ET):
                        ps_attn[(wi, e)] = psA.tile([128, PP], F32,
                                                    name=f"ps_attn_{wi}_{e}",
                                                    tag=f"attn_{wi}_{e}")
                for kq in range(KT4):
                    # per-ks descriptors + persistent tiles: fine-grained
                    # arrival keeps PE catch-up stalls under the 3.4us HAM
                    # re-throttle window, and no buffer-reuse WAR waits (a
                    # waiting DMA trigger head-of-line blocks its engine)
                    ptq_t = pqp.tile([128, 4, PP], F16, name=f"ptq_{kq}", tag="ptq")
                    p8 = pp_.tile([128, 4, PP], F8, name=f"pt8_{kq}", tag=f"pt8_{kq}")
                    for ks in range(4):
                        nc.scalar.dma_start(
                            out=ptq_t[:, ks, :],
                            in_=ptq_d[kq * 128:(kq + 1) * 128,
                                      ks * PP:(ks + 1) * PP])
                        nc.vector.tensor_scalar_mul(p8[:, ks, :],
                                                    ptq_t[:, ks, :], SF_P)
                    pt8_sb.append(p8)
                    tf_t = wp.tile([128, 4, 2 * E], F16, name=f"tf_{kq}", tag=f"w{kq}")
                    eng = nc.sync if kq % 2 == 0 else nc.gpsimd
                    for ks in range(4):
                        eng.dma_start(
                            out=tf_t[:, ks, :],
                            in_=tf_d[kq * 128:(kq + 1) * 128,
                                     ks * 2 * E:(ks + 1) * 2 * E])
                    for ks in range(4):
                        k = kq * 4 + ks
                        for wi in (0, 1):
                            for e in range(ET):
                                nc.tensor.matmul(
                                    ps_attn[(wi, e)][:, :],
                                    tf_t[:, ks, wi * E + e * 128: wi * E + (e + 1) * 128],
                                    ptq_t[:, ks, :],
                                    start=(k == 0), stop=(k == KT - 1))
                proj_sb = {}
                for e in range(ET):
                    for wi in (0, 1):
                        sb = pp_.tile([128, PP], F16, name=f"proj_{wi}_{e}",
                                      tag=f"proj_{wi}_{e}")
                        if bias_sb[wi]:
                            nc.scalar.activation(sb[:, :], ps_attn[(wi, e)][:, :],
                                                 mybir.ActivationFunctionType.Identity,
                                                 bias=bias_sb[wi][e][:, :], scale=1.0)
                        elif wi == 0:
                            nc.scalar.copy(sb[:, :], ps_attn[(wi, e)][:, :])
                        else:
                            nc.vector.tensor_copy(sb[:, :], ps_attn[(wi, e)][:, :])
                        proj_sb[(wi, e)] = sb

            # ---- scores + softmax per (sample, p-chunk); fold fp8 unscale ----
            wgt = {}
            with tc.tile_pool(name="psB", bufs=1, space="PSUM") as psB:
                sps_t = {}
                for (s, pc) in GRP:
                    sps_t[(s, pc)] = psB.tile([128, P], F32,
                                              name=f"ps_sc_{s}_{pc}", tag="sc", bufs=4)
                for e in range(ET):
                    for (s, pc) in GRP:
                        col = s * P + pc * 128
                        nc.tensor.matmul(sps_t[(s, pc)][:, :],
                                         proj_sb[(0, e)][:, col:col + 128],
                                         proj_sb[(1, e)][:, s * P:(s + 1) * P],
                                         start=(e == 0), stop=(e == ET - 1))
                for (s, pc) in GRP:
                    sps = sps_t[(s, pc)]
                    mx = smp.tile([128, 1], F32, name=f"mx_{s}_{pc}", tag="mx")
                    nc.vector.tensor_reduce(out=mx[:, :], in_=sps[:, :],
                                            axis=mybir.AxisListType.X, op=mybir.AluOpType.max)
                    ngm = smp.tile([128, 1], F32, name=f"ngm_{s}_{pc}", tag="ngm")
                    nc.vector.tensor_scalar_mul(ngm[:, :], mx[:, :], -1.0)
                    ex = smp.tile([128, P], F32, name=f"ex_{s}_{pc}", tag="ex")
                    ssum = smp.tile([128, 1], F32, name=f"ssum_{s}_{pc}", tag="ssum")
                    nc.scalar.activation(ex[:, :], sps[:, :], mybir.ActivationFunctionType.Exp,
                                         bias=ngm[:, :], scale=1.0, accum_out=ssum[:, :])
                    ssc = smp.tile([128, 1], F32, name=f"ssc_{s}_{pc}", tag="ssc")
                    nc.vector.tensor_scalar_mul(ssc[:, :], ssum[:, :], float(SF_P * SF_G))
                    rec = smp.tile([128, 1], F32, name=f"rec_{s}_{pc}", tag="rec")
                    nc.vector.reciprocal(rec[:, :], ssc[:, :])
                    wt_ = pp_.tile([128, P], F32, name=f"wgt_{s}_{pc}", tag=f"wgt_{s}_{pc}")
                    nc.vector.tensor_scalar_mul(wt_[:, :], ex[:, :], rec[:, :])
                    wgt[(s, pc)] = wt_

            # ---- g projection (fp8 DoubleRow) + gating + residual ----
            GW_ENG = [nc.sync, nc.gpsimd, nc.sync, nc.gpsimd,
                      nc.sync, nc.gpsimd, nc.sync, nc.gpsimd]
            OUT_ENG = [nc.scalar, nc.sync, nc.gpsimd, None]
            with tc.tile_pool(name="psC", bufs=1, space="PSUM") as psC:
                for dp in range(DP):
                    en16 = {}
                    pn16 = {}
                    for gi, (s, pc) in enumerate(GRP):
                        en16[gi] = ep.tile([128, 1024], F16, name=f"en_{dp}_{gi}", tag="en")
                        pn16[gi] = pnp.tile([128, 1024], F16, name=f"pn_{dp}_{gi}", tag="pn")
                        row = s * P + pc * 128
                        nc.scalar.dma_start(
                            out=pn16[gi][:, :],
                            in_=pnat_d[row:row + 128, dp * 1024:(dp + 1) * 1024])

                    def gate(dch, gi, s, pc, g_ps, dp=dp, en16=en16, pn16=pn16):
                        if gb_sb is not None:
                            nc.vector.tensor_add(
                                g_ps[:, :], g_ps[:, :],
                                gb_sb[0:1, dch * 512:(dch + 1) * 512]
                                .partition_broadcast(128))
                        base = (dch % 2) * 512
                        row = s * P + pc * 128
                        final = (dp == DP - 1 and gi == len(GRP) - 1)
                        if final and dch % 2 == 1:
                            # last tile of the kernel: finest-grain gating so
                            # each 256-col chunk's write starts immediately
                            for q in range(2):
                                a = base + q * 256
                                nc.vector.tensor_mul(en16[gi][:, a:a + 256],
                                                     g_ps[:, q * 256:(q + 1) * 256],
                                                     wgt[(s, pc)][:, :])
                                nc.vector.tensor_add(en16[gi][:, a:a + 256],
                                                     en16[gi][:, a:a + 256],
                                                     pn16[gi][:, a:a + 256])
                                eng = nc.sync if q == 0 else nc.gpsimd
                                eng.dma_start(
                                    out=out_d[row:row + 128,
                                              dp * 1024 + a:dp * 1024 + a + 256],
                                    in_=en16[gi][:, a:a + 256])
                            return
                        nc.vector.tensor_mul(en16[gi][:, base:base + 256],
                                             g_ps[:, 0:256], wgt[(s, pc)][:, :])
                        nc.vector.tensor_mul(en16[gi][:, base + 256:base + 512],
                                             g_ps[:, 256:512], wgt[(s, pc)][:, :])
                        nc.vector.tensor_add(en16[gi][:, base:base + 512],
                                             en16[gi][:, base:base + 512],
                                             pn16[gi][:, base:base + 512])
                        if final and dch % 2 == 0:
                            # write the first half of the final tile early
                            nc.scalar.dma_start(
                                out=out_d[row:row + 128, dp * 1024:dp * 1024 + 512],
                                in_=en16[gi][:, 0:512])
                        elif dch % 2 == 1:
                            eng = OUT_ENG[gi] if dp == DP - 1 else nc.scalar
                            eng.dma_start(
                                out=out_d[row:row + 128, dp * 1024:(dp + 1) * 1024],
                                in_=en16[gi][:, :])

                    # one 512KB descriptor per (dp, kq) covers both sub-rounds:
                    # tile free layout (sub, ks, n); rings alternate by kq
                    gts = []
                    for kq in range(KT4):
                        gt = gp.tile([128, 8, 512], F8, name=f"gt_{dp}_{kq}", tag="gt")
                        eng = nc.sync if kq % 2 == 0 else nc.gpsimd
                        eng.dma_start(
                            out=gt[:, :, :],
                            in_=gw8_d[kq * 128:(kq + 1) * 128,
                                      dp * 4096:(dp + 1) * 4096])
                        gts.append(gt)
                    for sub in range(2):
                        dch = dp * 2 + sub
                        last = (dp == DP - 1 and sub == 1)
                        if not last:
                            gps = {}
                            for gi, (s, pc) in enumerate(GRP):
                                gps[gi] = psC.tile([128, 512], F32,
                                                   name=f"ps_g_{dch}_{gi}", tag="g", bufs=6)
                            for kq in range(KT4):
                                for gi, (s, pc) in enumerate(GRP):
                                    col = s * P + pc * 128
                                    for pr in range(2):
                                        nc.tensor.matmul(
                                            gps[gi][:, :],
                                            pt8_sb[kq][:, 2 * pr:2 * pr + 2, col:col + 128],
                                            gts[kq][:, sub * 4 + 2 * pr:sub * 4 + 2 * pr + 2, :],
                                            start=(kq == 0 and pr == 0),
                                            stop=(kq == KT4 - 1 and pr == 1),
                                            perf_mode=DR)
                            for gi, (s, pc) in enumerate(GRP):
                                gate(dch, gi, s, pc, gps[gi])
                        else:
                            # final round k-inner per group: gating/writes of
                            # earlier groups overlap the remaining matmuls
                            for gi, (s, pc) in enumerate(GRP):
                                col = s * P + pc * 128
                                g_ps = psC.tile([128, 512], F32,
                                                name=f"ps_g_{dch}_{gi}", tag="g", bufs=6)
                                for kq in range(KT4):
                                    for pr in range(2):
                                        nc.tensor.matmul(
                                            g_ps[:, :],
                                            pt8_sb[kq][:, 2 * pr:2 * pr + 2, col:col + 128],
                                            gts[kq][:, sub * 4 + 2 * pr:sub * 4 + 2 * pr + 2, :],
                                            start=(kq == 0 and pr == 0),
                                            stop=(kq == KT4 - 1 and pr == 1),
                                            perf_mode=DR)
                                gate(dch, gi, s, pc, g_ps)

    nc.compile()
    _built[key] = nc
    return nc


def kernel(**inputs):
    global LAST_RESULTS
    x = np.ascontiguousarray(inputs["x"], dtype=np.float32)
    tw = np.asarray(inputs["theta_w"], dtype=np.float32)
    fw = np.asarray(inputs["f_w"], dtype=np.float32)
    gw = np.asarray(inputs["g_w"], dtype=np.float32)
    tb = np.asarray(inputs["theta_b"], dtype=np.float32)
    fb = np.asarray(inputs["f_b"], dtype=np.float32)
    gb = np.asarray(inputs["g_b"], dtype=np.float32)
    scale = float(np.asarray(inputs["scale"], dtype=np.float32).reshape(-1)[0])

    with_tb = bool(np.any(tb))
    with_fb = bool(np.any(fb))
    with_gb = bool(np.any(gb))
    nc = _build(with_tb, with_fb, with_gb)

    F8NP = mybir.dt.np(F8)
    # patchify: [B,C,H,W] -> [B,P,D] with D ordered (c, u, v)
    p = x.reshape(B, C, NPS, PH, NPS, PW).transpose(0, 2, 4, 1, 3, 5).reshape(B, P, D)
    tf16 = np.concatenate([tw, fw], axis=1).astype(np.float16)
    # k-quad interleave: row kq*128+part, col ks*1024 + wcol
    tf16 = np.ascontiguousarray(
        tf16.reshape(KT4, 4, 128, 2 * E).transpose(0, 2, 1, 3)
            .reshape(KT4 * 128, 4 * 2 * E))
    # gw8: [kq, ks, part, dch, n] -> [kq, part, dch, ks, n]
    gq = np.clip(gw * (scale * SF_G), -240.0, 240.0).astype(F8NP)
    gw8 = np.ascontiguousarray(
        gq.reshape(KT4, 4, 128, DCH, 512).transpose(0, 2, 3, 1, 4)
          .reshape(KT4 * 128, DCH * 2048))
    in_maps = []
    for ci in range(NCORES):
        p2 = p[ci * SPC:(ci + 1) * SPC]                      # [SPC, P, D]
        pnat = p2.reshape(PP, D).astype(np.float16)
        pT = p2.transpose(2, 0, 1).reshape(D, PP)
        ptq = np.ascontiguousarray(
            pT.reshape(KT4, 4, 128, PP).transpose(0, 2, 1, 3)
              .reshape(KT4 * 128, 4 * PP)).astype(np.float16)
        m = {"ptq": ptq, "pnat": pnat, "tf": tf16, "gw8": gw8}
        if with_tb:
            m["tb"] = np.ascontiguousarray(tb.reshape(E, 1))
        if with_fb:
            m["fb"] = np.ascontiguousarray(fb.reshape(E, 1))
        if with_gb:
            m["gb"] = np.ascontiguousarray((gb * (scale * SF_P * SF_G)).reshape(1, D))
        in_maps.append(m)

    res = run_bass_kernel_spmd(nc, in_maps, core_ids=list(range(NCORES)))
    LAST_RESULTS = res
    o = np.concatenate([np.asarray(res.results[ci]["out"], dtype=np.float32)
                        .reshape(SPC, P, D)
                        for ci in range(NCORES)], axis=0)     # [B, P, D]
    img = (o.reshape(B, NPS, NPS, C, PH, PW)
            .transpose(0, 3, 1, 4, 2, 5)
            .reshape(B, C, H, W))
    return np.ascontiguousarray(img, dtype=np.float32)
